# revision 42
# baseline (speedup 1.0000x reference)
"""Trainium2 Bass kernel for nn_AutoregressiveHead.

Reference computation (L=32 heads, D=1024, H=512, B=8192, P=2 parents):
    base = einsum('bd,ldh->blh', x, Wx)
    parents = y[:, parent_idx]                  # [B, L, P]
    pc = einsum('blp,lph->blh', parents, Wp)
    h = relu(base + pc + b1)
    out = einsum('blh,lh->bl', h, W2) + b2      # [B, L]

Strategy (data-parallel over B across 8 NeuronCores, weights replicated):
  * Per core: B_core=1024 batch rows.  The dominant compute is the 32
    per-head matmuls x @ Wx[l] ([1024,1024]@[1024,512], bf16, fp32 PSUM).
  * The parent gather + b1 are folded into one extra K<=128 matmul:
    y_aug = [y^T; ones; 0-pad] (K=128 padded), W_aug[l] = [Wp scattered to
    dense label rows; b1[l]; 0] so  x@Wx[l] + y_aug@W_aug[l] = base+pc+b1.
  * |W2[l,h]| is folded into Wx/W_aug columns on the host, and columns are
    permuted so positive-sign-W2 columns come first.  Then
        out[b,l] = sum_pos relu(z') - sum_neg relu(z') + b2[l]
    which the Scalar engine computes with activation(Relu, accum_out=...)
    over the two column ranges -- no second matmul stage, h never stored.
  * PSUM tile [128, 512] per head; l processed in groups of 4 with the Wx
    slab for the group resident in SBUF (triple-buffered across groups).

  * parent_mode="dve" (current default): the parent/bias term pc+b1 is
    NOT a PE matmul.  The DVE computes sc = y0*Wp0 + y1*Wp1 + b1 per
    (head, bt) from partition-broadcast tables (2 tensor_scalar + 2
    tensor_tensor, bf16, overlapped with the matmuls) and then adds sc
    into the PSUM tile (1 fp32 tensor_tensor) after the k7 matmul,
    before the ScalarE drain.  This removes the 4 per-group y-strip
    matmul streams (~30-45us of PE time).

Measured on HW (8 axon TRN2 cores, For_i hw-loop delta timing, which is
robust to the ~2-4ms per-call axon dispatch floor -- do NOT trust
repeats=1-vs-5 deltas, repeats=1 is dispatch-bound):
  * this kernel (parent_mode=dve):  ~551-556 us/iter, rel err 0.00242
  * parent_mode=pe (old baseline):  ~583-586 us/iter
  * noparent (main matmuls only):   ~553 us/iter  -> parent cost on PE
    was ~30us (the 4 tile_position strip-mms overlap ~2x; 2 strips cost
    the same as 4, so a PE/DVE hybrid split does NOT help - measured).
Microbenchmarked PE facts (exp_mm.py): 512-wide bf16 matmul issues at
~262 ns steady (=2.0 GHz effective, not 2.4; cost is PROPORTIONAL to
rhs width - 256-wide is ~133ns - so it is a clock/column limit, not
LDWEIGHTS or fixed overhead; fp8 rhs streams at the SAME 262ns, so the
limit is column rate, not SBUF bytes).  Main-matmul floor = 2048 mm x
262ns = 537us/core; this kernel is ~3% above it.
Tried and rejected (measured A/B): stage2 variants (nostage2 == base,
the ScalarE drain is fully hidden), GpSimd for the SBUF adds (gpsimd
tensor_tensor is ~10x slower than DVE -> 793us), PE/DVE hybrid parent
(586us), pc pool bufs=16 (no gain), fp8 DoubleRow matmuls (e4m3 both
sides gives 3.7% rel err vs the 2e-2 gate; one-sided 2.6%; any hi-lo
correction term costs a full extra matmul pass, killing the 2x win).
GpSimd cannot access PSUM (walrus verifier).
Round-2 A/B results (all correct, rel 0.00242, all REJECTED on time):
  * dvew (one [128,G,H] 4-bank psum tile/bt + merged wide DVE ops):
    573us - coarser psum granularity stalls the drain pipeline.
  * dves (ACT-seeded b1 Copy into PSUM + all-start=False matmuls after
    8 warmup mms): 575us - the seed Copy lands on the PE critical path
    (k0 waits on ACT) costing more than the saved DVE op.  NOTE the
    has_written semantics DID validate on HW: engine (ACT) writes to
    PSUM do not clear has_written, so start=False matmuls accumulate
    correctly onto engine-written values after a one-time per-bank
    start=True warmup; rel err was bit-identical 0.002421 (a dropped
    b1 would have shown ~0.01).
  * interleaving each head's psum-add right after its k7 matmul
    (instead of after all 4 heads): kept, ~551 vs ~556us (within
    noise, no downside).
Round-3 A/B (all correct):
  * act_mul=True (one of the two per-unit tensor_scalar muls moved to
    ScalarE via activation(Copy, scale=yc-AP)): 556 vs 556us - a wash;
    left OFF in the default path (flag kept for future rebalancing).
  * wx slab prefetch bufs 3->4: 548 vs 556us median - kept (deeper
    slab prefetch across lg boundaries).
  * pw pool bufs 3->2 + pc bufs 8->12 (trade SBUF from the small table
    prefetch to the DVE lookahead): ~547us median, best round 537
    (= the PE floor) - kept, equal-or-better and frees nothing else.
Remaining known slack: ~10us DVE/PE co-criticality (DVE ~7.9us/bt vs
PE ~8.4us/bt) and ~15-20us cold-run prologue (xt+slab0 DMA serial).
"""

import os
import numpy as np
import ml_dtypes

import bass_rust
import concourse.bass as bass
import concourse.tile as tile
from concourse import mybir
from concourse.vector_clock import ScopedClock

BF16 = ml_dtypes.bfloat16

N_CORES = 8
B, D, H, L = 8192, 1024, 512, 32
B_CORE = B // N_CORES          # 1024
PPART = 128                    # partition size
KT = D // PPART                # 8 k-tiles over D
G = 4                          # heads per PSUM group
N_LG = L // G                  # 8 groups


class SplitDrainTileContext(tile.TileContext):
    """The walrus build in this container rejects >1 sem waits on the tail
    Drain ("Too many sync wait commands").  Redistribute the global-clock
    waits onto single-wait nops preceding the drain."""

    def _drain_and_barrier(self, tick_clock, wait_clock):
        probe = self.nc.sync.nop(nofuse=True)
        wait_clock.add_sem_waits(
            probe.ins, ScopedClock({None: tick_clock.global_clock})
        )
        si = probe.ins.sync_info
        waits = list(si.on_wait) if si is not None and si.on_wait else []
        if len(waits) > 1:
            si.on_wait = waits[:1]
            for w in waits[1:]:
                n = self.nc.sync.nop(nofuse=True)
                n.ins.sync_info = bass_rust.SyncInfo(on_wait=[w], on_update=[])
        self.nc.sync.drain()
        self.nc.all_engine_barrier()
        assert self.sems is not None
        popped = self.nc._tile_sem_poison_stack.pop()
        assert popped is self._sem_poison
        self.nc.clear_and_free_semaphores(list(self.sems.allocated().values()))
        self.nc.all_engine_barrier()


def split_multi_waits(nc, max_waits: int = 1):
    """This container's walrus rejects instructions carrying more than one
    sem-wait ("Too many sync wait commands").  Hoist extra waits onto
    single-wait NoOps inserted just before the instruction on the same
    engine (engine order is preserved; sems are monotonic, so waiting
    earlier on the same engine is equivalent)."""
    uid = 0
    for f in nc.m.functions:
        for blk in f.blocks:
            insts = blk.instructions
            new = []
            for inst in insts:
                si = inst.sync_info
                waits = list(si.on_wait) if si is not None and si.on_wait else []
                if len(waits) > max_waits:
                    for w in waits[:-max_waits]:
                        nop = mybir.InstNoOp(
                            name=f"splitw-{uid}", engine=inst.engine,
                            ins=[], outs=[],
                        )
                        uid += 1
                        nop.sync_info = bass_rust.SyncInfo(
                            on_wait=[w], on_update=[]
                        )
                        nc.register_instruction(nop, overwrite=True)
                        new.append(nop)
                    si.on_wait = waits[-max_waits:]
                new.append(inst)
            insts[:] = new


def build_program(n_bt: int, k_pos, repeats: int = 1, ytile: bool = True,
                  stage2: str = "act2", hw_loop: bool = False,
                  parent: bool = True, parent_mode: str = "pe",
                  pool_add: bool = True, act_mul: bool = False):
    """Build the per-core Bass program.

    n_bt: number of 128-row batch tiles per core (8 for the real problem).
    k_pos: per-head split point -- columns [0:k_pos[l]) carry W2>=0,
           [k_pos[l]:H) carry W2<0 (after the host-side permutation).
    repeats: re-emit the whole compute `repeats` times (timing builds).
    ytile: pack the 4 per-head parent/bias matmuls of a group into
           concurrent 32-row-strip matmuls via tile_position (the strips
           execute in parallel in the PE array).
    """
    f32 = mybir.dt.float32
    bf16 = mybir.dt.bfloat16
    bc = n_bt * PPART

    nc = bass.Bass("TRN2", target_bir_lowering=False, debug=False,
                   num_devices=N_CORES)

    xT_d = nc.dram_tensor("xT", [D, bc], bf16, kind="ExternalInput")
    # wxp layout groups G heads contiguously per (lg, k) for 512KB DMAs
    wxp_d = nc.dram_tensor("wxp", [N_LG, KT, G, PPART, H], bf16,
                           kind="ExternalInput")
    b2r_d = nc.dram_tensor("b2r", [PPART, L], f32, kind="ExternalInput")
    if parent_mode in ("dve", "dves", "dvew", "hybrid"):
        # compact parent/bias tables (device-side partition broadcast)
        pw_shape = [N_LG, 3, G, H] if parent_mode == "dvew" else [N_LG, G, 3, H]
        pw_d = nc.dram_tensor("pw", pw_shape, bf16, kind="ExternalInput")
        yc_d = nc.dram_tensor("yc", [PPART, n_bt * L * 2], f32,
                              kind="ExternalInput")
        if parent_mode == "hybrid":
            yp_d = nc.dram_tensor("yp", [N_LG, PPART, bc], bf16,
                                  kind="ExternalInput")
            wp_d = nc.dram_tensor("wp", [N_LG, PPART, H], bf16,
                                  kind="ExternalInput")
    elif ytile:
        # per (lg): strip g holds [y_par0; y_par1; ones; 0...] for head lg*G+g
        yp_d = nc.dram_tensor("yp", [N_LG, PPART, bc], bf16,
                              kind="ExternalInput")
        wp_d = nc.dram_tensor("wp", [N_LG, PPART, H], bf16,
                              kind="ExternalInput")
    else:
        yp_d = nc.dram_tensor("ya", [PPART, bc], bf16, kind="ExternalInput")
        wp_d = nc.dram_tensor("wpa", [PPART, L, H], bf16,
                              kind="ExternalInput")
    out_d = nc.dram_tensor("out", [bc, L], f32, kind="ExternalOutput")

    with SplitDrainTileContext(nc) as tc:
        with (
            tc.tile_pool(name="const", bufs=1) as const_pool,
            tc.tile_pool(name="wx", bufs=4) as wx_pool,
            tc.tile_pool(name="psum",
                         bufs=(2 if parent_mode == "dvew" else 8),
                         space="PSUM") as psum_pool,
            tc.tile_pool(name="scratch", bufs=4) as scratch_pool,
            tc.tile_pool(name="outp", bufs=4) as out_pool,
            tc.tile_pool(name="pw", bufs=2) as pw_pool,
            tc.tile_pool(name="pc",
                         bufs=(4 if parent_mode == "dvew" else 12)) as pc_pool,
        ):
            # --- resident tensors ---
            xt_sb = const_pool.tile([PPART, KT, bc], bf16, tag="xt")
            nc.sync.dma_start(
                xt_sb[:], xT_d.ap().rearrange("(kt p) b -> p kt b", p=PPART)
            )
            if parent_mode in ("dve", "dves", "dvew", "hybrid"):
                yc_sb = const_pool.tile([PPART, n_bt * L * 2], f32, tag="yc")
                nc.sync.dma_start(yc_sb[:], yc_d.ap())
                if parent_mode == "hybrid":
                    yp_sb = const_pool.tile([PPART, N_LG, bc], bf16, tag="yp")
                    nc.sync.dma_start(
                        yp_sb[:], yp_d.ap().rearrange("lg p b -> p lg b")
                    )
                    wp_sb = const_pool.tile([PPART, N_LG, H], bf16, tag="wp")
                    nc.sync.dma_start(
                        wp_sb[:], wp_d.ap().rearrange("lg p h -> p lg h")
                    )
            elif ytile:
                yp_sb = const_pool.tile([PPART, N_LG, bc], bf16, tag="yp")
                nc.sync.dma_start(
                    yp_sb[:], yp_d.ap().rearrange("lg p b -> p lg b")
                )
                wp_sb = const_pool.tile([PPART, N_LG, H], bf16, tag="wp")
                nc.sync.dma_start(
                    wp_sb[:], wp_d.ap().rearrange("lg p h -> p lg h")
                )
            else:
                yp_sb = const_pool.tile([PPART, bc], bf16, tag="yp")
                nc.sync.dma_start(yp_sb[:], yp_d.ap())
                wp_sb = const_pool.tile([PPART, L, H], bf16, tag="wp")
                nc.sync.dma_start(wp_sb[:], wp_d.ap())
            b2_sb = const_pool.tile([PPART, L], f32, tag="b2")
            nc.sync.dma_start(b2_sb[:], b2r_d.ap())

            pos_sb = const_pool.tile([PPART, n_bt * L], f32, tag="pos")
            neg_sb = const_pool.tile([PPART, n_bt * L], f32, tag="neg")
            nc.vector.memset(pos_sb[:], 0.0)
            nc.vector.memset(neg_sb[:], 0.0)
            zero_sb = const_pool.tile([PPART, H], bf16, tag="zero")
            nc.vector.memset(zero_sb[:], 0.0)

            if parent_mode == "dves":
                warm = [
                    psum_pool.tile([PPART, H], f32, tag="ps", name="wm")
                    for _ in range(8)
                ]
                for w in warm:
                    nc.tensor.matmul(
                        w[:], lhsT=xt_sb[:, 0, 0:PPART],
                        rhs=xt_sb[:, 0, 0:H],
                        start=True, stop=True,
                    )

            import contextlib

            if hw_loop:
                rep_ctx = tc.For_i(0, repeats)
                rep_range = [0]
            else:
                rep_ctx = contextlib.nullcontext()
                rep_range = range(repeats)

            with rep_ctx:
              for _rep in rep_range:
                for lg in range(N_LG):
                    wx_sb = wx_pool.tile([PPART, KT, G, H], bf16, tag="wx")
                    for k in range(KT):
                        nc.sync.dma_start(
                            wx_sb[:, k, :, :],
                            wxp_d.ap()[lg, k].rearrange("g p h -> p g h"),
                        )
                    if parent_mode in ("dve", "dves", "dvew", "hybrid"):
                        shp = ([PPART, 3, G, H] if parent_mode == "dvew"
                               else [PPART, G, 3, H])
                        pw_sb = pw_pool.tile(shp, bf16, tag="pw")
                        nc.sync.dma_start(
                            pw_sb[:],
                            pw_d.ap()[lg].partition_broadcast(PPART),
                        )
                    if parent_mode in ("dve", "dves", "dvew"):
                        dve_heads = tuple(range(G))
                    elif parent_mode == "hybrid":
                        dve_heads = (2, 3)
                    else:
                        dve_heads = ()
                    for bt in range(n_bt):
                        if parent_mode == "dvew":
                            ps_big = psum_pool.tile([PPART, G, H], f32,
                                                    tag="psb", name="ps_big")
                            ps = [ps_big[:, g, :] for g in range(G)]
                        else:
                            ps = [
                                psum_pool.tile([PPART, H], f32, tag="ps",
                                               name="ps")
                                for _ in range(G)
                            ]
                        pcs = {}
                        if parent_mode == "dvew":
                            # merged wide DVE ops over all G heads
                            sc3 = pc_pool.tile([PPART, G, H], bf16, tag="pc",
                                               name="sc3")
                            sc4 = pc_pool.tile([PPART, G, H], bf16, tag="pc4",
                                               name="sc4")
                            for g in range(G):
                                head = lg * G + g
                                c0 = (bt * L + head) * 2
                                nc.vector.tensor_scalar(
                                    sc3[:, g, :], pw_sb[:, 0, g, :],
                                    yc_sb[:, c0:c0 + 1], None,
                                    mybir.AluOpType.mult,
                                )
                                nc.vector.tensor_scalar(
                                    sc4[:, g, :], pw_sb[:, 1, g, :],
                                    yc_sb[:, c0 + 1:c0 + 2], None,
                                    mybir.AluOpType.mult,
                                )
                            nc.vector.tensor_tensor(
                                sc3[:], sc3[:], sc4[:], mybir.AluOpType.add,
                            )
                            nc.vector.tensor_tensor(
                                sc3[:], sc3[:], pw_sb[:, 2, :, :],
                                mybir.AluOpType.add,
                            )
                        elif dve_heads:
                            # pc+b1 computed on DVE (overlaps the matmuls),
                            # then added into PSUM post-matmul.
                            seed = parent_mode == "dves"
                            for g in dve_heads:
                                head = lg * G + g
                                c0 = (bt * L + head) * 2
                                sc3 = pc_pool.tile([PPART, H], bf16, tag="pc",
                                                   name="sc3")
                                sc4 = pc_pool.tile([PPART, H], bf16, tag="pc4",
                                                   name="sc4")
                                if act_mul:
                                    # per-partition-scalar multiply on the
                                    # Scalar engine (activation scale AP)
                                    # to offload the co-critical DVE
                                    nc.scalar.activation(
                                        sc3[:], pw_sb[:, g, 0, :],
                                        mybir.ActivationFunctionType.Copy,
                                        scale=yc_sb[:, c0:c0 + 1],
                                    )
                                else:
                                    nc.vector.tensor_scalar(
                                        sc3[:], pw_sb[:, g, 0, :],
                                        yc_sb[:, c0:c0 + 1], None,
                                        mybir.AluOpType.mult,
                                    )
                                nc.vector.tensor_scalar(
                                    sc4[:], pw_sb[:, g, 1, :],
                                    yc_sb[:, c0 + 1:c0 + 2], None,
                                    mybir.AluOpType.mult,
                                )
                                nc.vector.tensor_tensor(
                                    sc3[:], sc3[:], sc4[:],
                                    mybir.AluOpType.add,
                                )
                                if not seed:
                                    nc.vector.tensor_tensor(
                                        sc3[:], sc3[:], pw_sb[:, g, 2, :],
                                        mybir.AluOpType.add,
                                    )
                                pcs[g] = sc3
                        # g-outer / k-inner: lhsT changes every matmul, which
                        # measures ~1.5x faster per-mm than same-lhsT runs
                        # (PE weight-load pipelining quirk), and lets the
                        # ACT drain of ps[g] start before the bt finishes.
                        interleave = parent_mode in ("dve", "dves") and parent
                        seed = parent_mode == "dves"
                        for g in range(G):
                            last_mm = (not parent) or g in dve_heads
                            if seed:
                                # overwrite the bank with b1 (fp32); the
                                # start=False matmuls accumulate on top
                                # (has_written set once by the warmups)
                                nc.scalar.activation(
                                    ps[g][:], pw_sb[:, g, 2, :],
                                    mybir.ActivationFunctionType.Copy,
                                )
                            for k in range(KT):
                                nc.tensor.matmul(
                                    ps[g][:],
                                    lhsT=xt_sb[:, k,
                                               bt * PPART:(bt + 1) * PPART],
                                    rhs=wx_sb[:, k, g, :],
                                    start=(k == 0 and not seed),
                                    stop=(last_mm and k == KT - 1),
                                )
                            if interleave:
                                # add pc+b1 right after this head's k7 so
                                # the drain/bank-release starts earlier
                                nc.vector.tensor_tensor(
                                    ps[g][:], ps[g][:], pcs[g][:],
                                    mybir.AluOpType.add,
                                )
                        if not parent or interleave:
                            pass
                        elif parent_mode == "dvew":
                            nc.vector.tensor_tensor(
                                ps_big[:], ps_big[:], sc3[:],
                                mybir.AluOpType.add,
                            )
                        elif parent_mode in ("dve", "hybrid"):
                            for g in range(G):
                                if g in dve_heads:
                                    nc.vector.tensor_tensor(
                                        ps[g][:], ps[g][:], pcs[g][:],
                                        mybir.AluOpType.add,
                                    )
                                else:
                                    nc.tensor.matmul(
                                        ps[g][:],
                                        lhsT=yp_sb[32 * g:32 * (g + 1), lg,
                                                   bt * PPART:(bt + 1) * PPART],
                                        rhs=wp_sb[32 * g:32 * (g + 1), lg, :],
                                        start=False, stop=True,
                                        tile_position=(32 * g, 0),
                                    )
                        elif ytile:
                            for g in range(G):
                                nc.tensor.matmul(
                                    ps[g][:],
                                    lhsT=yp_sb[32 * g:32 * (g + 1), lg,
                                               bt * PPART:(bt + 1) * PPART],
                                    rhs=wp_sb[32 * g:32 * (g + 1), lg, :],
                                    start=False, stop=True,
                                    tile_position=(32 * g, 0),
                                )
                        else:
                            ya_lhs = yp_sb[:, bt * PPART:(bt + 1) * PPART]
                            for g in range(G):
                                nc.tensor.matmul(
                                    ps[g][:], lhsT=ya_lhs,
                                    rhs=wp_sb[:, lg * G + g, :],
                                    start=False, stop=True,
                                )
                        for g in range(G):
                            head = lg * G + g
                            kl = int(k_pos[head])
                            col = bt * L + head
                            if stage2 == "none":
                                continue
                            if stage2 == "fullacc":
                                # timing-only: single act instr, all 512 cols
                                sc = scratch_pool.tile([PPART, H], bf16,
                                                       tag="sc", name="sc")
                                nc.scalar.activation(
                                    sc[:],
                                    ps[g][:],
                                    mybir.ActivationFunctionType.Relu,
                                    accum_out=pos_sb[:, col:col + 1],
                                )
                                continue
                            if stage2 == "posonly":
                                # timing-only: single act instr, ~half cols
                                sc = scratch_pool.tile([PPART, H], bf16,
                                                       tag="sc", name="sc")
                                nc.scalar.activation(
                                    sc[:, :256],
                                    ps[g][:, :256],
                                    mybir.ActivationFunctionType.Relu,
                                    accum_out=pos_sb[:, col:col + 1],
                                )
                                continue
                            if stage2 == "gsplit":
                                # pos on ScalarE, neg on GpSimd/Pool engine
                                sc = scratch_pool.tile([PPART, H], bf16,
                                                       tag="sc", name="sc")
                                if kl > 0:
                                    nc.scalar.activation(
                                        sc[:, :kl], ps[g][:, :kl],
                                        mybir.ActivationFunctionType.Relu,
                                        accum_out=pos_sb[:, col:col + 1],
                                    )
                                if kl < H:
                                    nc.gpsimd.tensor_scalar(
                                        sc[:, kl:], ps[g][:, kl:],
                                        0.0, None, mybir.AluOpType.max,
                                        accum_out=neg_sb[:, col:col + 1],
                                    )
                                continue
                            if stage2 == "act2p":
                                # relu main-out written back to PSUM in
                                # place: faster ACT access path and no
                                # SBUF scratch writes contending with the
                                # PE's rhs streams
                                if kl > 0:
                                    nc.scalar.activation(
                                        ps[g][:, :kl], ps[g][:, :kl],
                                        mybir.ActivationFunctionType.Relu,
                                        accum_out=pos_sb[:, col:col + 1],
                                    )
                                if kl < H:
                                    nc.scalar.activation(
                                        ps[g][:, kl:], ps[g][:, kl:],
                                        mybir.ActivationFunctionType.Relu,
                                        accum_out=neg_sb[:, col:col + 1],
                                    )
                                continue
                            sc = scratch_pool.tile([PPART, H], bf16, tag="sc")
                            if kl > 0:
                                nc.scalar.activation(
                                    sc[:, :kl], ps[g][:, :kl],
                                    mybir.ActivationFunctionType.Relu,
                                    accum_out=pos_sb[:, col:col + 1],
                                )
                            if kl < H:
                                if stage2 == "ttr":
                                    # negative-sign range reduced on DVE
                                    # (tensor_tensor_reduce, as in the QR
                                    # kernel) to halve the ScalarE load:
                                    # accum = sum(max(z, 0))
                                    scd = scratch_pool.tile(
                                        [PPART, H], bf16, tag="scd")
                                    nc.vector.tensor_tensor_reduce(
                                        scd[:, kl:], ps[g][:, kl:],
                                        zero_sb[:, :H - kl], 1.0, 0.0,
                                        mybir.AluOpType.max,
                                        mybir.AluOpType.add,
                                        accum_out=neg_sb[:, col:col + 1],
                                    )
                                elif stage2 == "split":
                                    # negative-sign range reduced on DVE to
                                    # halve the ScalarE load
                                    nc.vector.tensor_scalar(
                                        sc[:, kl:], ps[g][:, kl:],
                                        0.0, None, mybir.AluOpType.max,
                                        op1=mybir.AluOpType.add,
                                        accum_out=neg_sb[:, col:col + 1],
                                    )
                                else:
                                    nc.scalar.activation(
                                        sc[:, kl:], ps[g][:, kl:],
                                        mybir.ActivationFunctionType.Relu,
                                        accum_out=neg_sb[:, col:col + 1],
                                    )

            # --- epilogue: out = pos - neg + b2 ---
            for bt in range(n_bt):
                o = out_pool.tile([PPART, L], f32, tag="o")
                nc.vector.tensor_tensor(
                    o[:], pos_sb[:, bt * L:(bt + 1) * L],
                    neg_sb[:, bt * L:(bt + 1) * L], mybir.AluOpType.subtract,
                )
                nc.vector.tensor_tensor(
                    o[:], o[:], b2_sb[:], mybir.AluOpType.add,
                )
                nc.sync.dma_start(out_d.ap()[bt * PPART:(bt + 1) * PPART, :], o[:])

    split_multi_waits(nc)
    return nc


def prep_host(x, y, Wx, Wp, b1, W2, b2, parent_idx, n_bt: int = 8,
              ytile: bool = True, parent_mode: str = "pe"):
    """Host-side data prep.  Returns (in_maps per core, k_pos)."""
    x = np.asarray(x, np.float32)
    y = np.asarray(y, np.float32)
    Wx = np.asarray(Wx, np.float32)
    Wp = np.asarray(Wp, np.float32)
    b1 = np.asarray(b1, np.float32)
    W2 = np.asarray(W2, np.float32)
    b2 = np.asarray(b2, np.float32)
    parent_idx = np.asarray(parent_idx)
    NP = parent_idx.shape[1]

    bc = n_bt * PPART

    # |W2| folding + sign-partition permutation of the H axis (per head)
    s = np.abs(W2)                       # [L, H]
    k_pos = np.zeros(L, np.int64)
    perm = np.zeros((L, H), np.int64)
    for l in range(L):
        posm = W2[l] >= 0
        p_idx = np.concatenate([np.where(posm)[0], np.where(~posm)[0]])
        perm[l] = p_idx
        k_pos[l] = int(posm.sum())

    # wxp[lg, kt, g, p, h'] = Wx[l, kt*128+p, perm[l,h']] * s[l, perm[l,h']]
    wxp = np.empty((N_LG, KT, G, PPART, H), BF16)
    for l in range(L):
        m = (Wx[l] * s[l][None, :])[:, perm[l]]          # [D, H]
        wxp[l // G, :, l % G] = m.reshape(KT, PPART, H).astype(BF16)

    b2r = np.broadcast_to(b2[None, :], (PPART, L)).astype(np.float32).copy()

    if ytile:
        # wp[lg, 32g+j] = parent-j weights of head l=lg*G+g (scaled/permuted);
        # row 32g+NP = b1 row (pairs with the ones row of yp).
        wp = np.zeros((N_LG, PPART, H), np.float32)
        for l in range(L):
            lg, g = l // G, l % G
            for j in range(NP):
                wp[lg, 32 * g + j] = (Wp[l, j] * s[l])[perm[l]]
            wp[lg, 32 * g + NP] = (b1[l] * s[l])[perm[l]]
        wp = wp.astype(BF16)
    else:
        # dense label-indexed parent weights + b1 via ones row (K=128 padded)
        Wfull = np.zeros((L, L, H), np.float32)
        for l in range(L):
            for p in range(NP):
                Wfull[l, parent_idx[l, p]] += Wp[l, p]
        wpa = np.zeros((PPART, L, H), np.float32)
        for l in range(L):
            wpa[:L, l, :] = (Wfull[l] * s[l][None, :])[:, perm[l]]
            wpa[L, l, :] = (b1[l] * s[l])[perm[l]]
        wpa = wpa.astype(BF16)

    if parent_mode in ("dve", "dves", "dvew", "hybrid"):
        # pw[lg, :, g, 0/1, :] = scaled/permuted parent-j weights (replicated
        # across partitions); [..., 2, :] = scaled/permuted b1.
        pwc = np.zeros((N_LG, G, 3, H), np.float32)
        for l in range(L):
            lg, g = l // G, l % G
            for j in range(NP):
                pwc[lg, g, j] = (Wp[l, j] * s[l])[perm[l]]
            pwc[lg, g, 2] = (b1[l] * s[l])[perm[l]]
        if parent_mode == "dvew":
            pwc = pwc.transpose(0, 2, 1, 3)           # [N_LG, 3, G, H]
        pw = np.ascontiguousarray(pwc.astype(BF16))

    in_maps = []
    for c in range(N_CORES):
        xs = x[c * bc:(c + 1) * bc]                       # [bc, D]
        ys = y[c * bc:(c + 1) * bc]                       # [bc, L]
        xT = np.ascontiguousarray(xs.T).astype(BF16)      # [D, bc]
        m = {"xT": xT, "wxp": wxp, "b2r": b2r}
        if parent_mode in ("dve", "dves", "dvew", "hybrid"):
            # yc[p, ((bt*L)+l)*2+j] = y[c*bc + bt*128 + p, pid[l, j]]
            yg = ys[:, parent_idx]                        # [bc, L, 2]
            yc = yg.reshape(n_bt, PPART, L * NP).transpose(1, 0, 2)
            m["yc"] = np.ascontiguousarray(
                yc.reshape(PPART, n_bt * L * NP)).astype(np.float32)
            m["pw"] = pw
            if parent_mode == "hybrid":
                yp = np.zeros((N_LG, PPART, bc), np.float32)
                for l in range(L):
                    lg, g = l // G, l % G
                    for j in range(NP):
                        yp[lg, 32 * g + j] = ys[:, parent_idx[l, j]]
                    yp[lg, 32 * g + NP] = 1.0
                m["yp"] = yp.astype(BF16)
                m["wp"] = wp
        elif ytile:
            yp = np.zeros((N_LG, PPART, bc), np.float32)
            for l in range(L):
                lg, g = l // G, l % G
                for j in range(NP):
                    yp[lg, 32 * g + j] = ys[:, parent_idx[l, j]]
                yp[lg, 32 * g + NP] = 1.0
            m["yp"] = yp.astype(BF16)
            m["wp"] = wp
        else:
            ya = np.zeros((PPART, bc), np.float32)
            ya[:L] = ys.T
            ya[L] = 1.0
            m["ya"] = ya.astype(BF16)
            m["wpa"] = wpa
        in_maps.append(m)
    return in_maps, k_pos


def reference_host(x, y, Wx, Wp, b1, W2, b2, parent_idx):
    """numpy fp32 reference (for sim tests inside this module)."""
    base = np.einsum('bd,ldh->blh', x, Wx)
    parents = y[:, parent_idx]
    pc = np.einsum('blp,lph->blh', parents, Wp)
    h = np.maximum(base + pc + b1[None], 0.0)
    return np.einsum('blh,lh->bl', h, W2) + b2


_CACHE = {}

# parent/bias contribution path: "dve" computes pc+b1 on the Vector engine
# and adds it into PSUM, freeing ~30-45us of Tensor-engine streams vs the
# "pe" (y-strip matmul) path.
PARENT_MODE = "dve"


def kernel(x, y, Wx, Wp, b1, W2, b2, parent_idx):
    from concourse.bass_utils import run_bass_kernel_spmd

    x = np.asarray(x)
    n_bt = x.shape[0] // N_CORES // PPART
    in_maps, k_pos = prep_host(x, y, Wx, Wp, b1, W2, b2, parent_idx, n_bt=n_bt,
                               parent_mode=PARENT_MODE)

    key = (n_bt, PARENT_MODE, tuple(int(v) for v in k_pos))
    if key not in _CACHE:
        _CACHE[key] = build_program(n_bt, k_pos, parent_mode=PARENT_MODE)
    nc = _CACHE[key]

    res = run_bass_kernel_spmd(nc, in_maps, core_ids=list(range(N_CORES)))
    out = np.concatenate([res.results[c]["out"] for c in range(N_CORES)], axis=0)
    return out.astype(np.float32)

